# revision 1
# baseline (speedup 1.0000x reference)
"""Bipolar self-attention on 8 Trainium2 NeuronCores.

Sharding: data-parallel over batch (B=2 -> 2 groups of 4 cores), tensor-
parallel over heads within a group (16 heads -> 4 heads/core). Each core:
  - projects its head-slice of Q/K transposed ([c, n] layout) and V natural,
    with the bipolar transform (q-0.5)*2 and the 1/sqrt(Dh) score scale
    folded into the projection weights/biases host-side,
  - computes S^T = Kb Qb^T per head tile-by-tile, exponentiates (softmax
    without max subtraction -- scores are O(10), exp is safe in fp32),
  - multiplies P^T by V augmented with a ones column, so the softmax
    denominator falls out of the same matmul (row 64 of the accumulator),
  - normalizes and applies its slice of the output projection (row-parallel).
Host sums the 4 partial outputs per batch and adds the bias terms.

All matmuls run in float32r (full fp32 storage, reduced-precision multiply)
which is 4x faster than fp32 on the PE at moving-dim >= 256. Head pairs are
packed into disjoint PE row groups (partitions 0-63 / 64-127) so the K=64
QK^T matmuls of two heads run concurrently, and one ACTIVATE exponentiates
both heads' scores.
"""

import numpy as np

import concourse.bass as bass
import concourse.tile as tile
from concourse import bacc, mybir
from concourse.bass_utils import run_bass_kernel_spmd

D_MODEL = 1024
NHEAD = 16
HEAD_DIM = 64
B = 2
N = 2048
N_CORES = 8
HEADS_PER_CORE = NHEAD // (N_CORES // B)  # 4
C_LOC = HEADS_PER_CORE * HEAD_DIM  # 256

F32 = mybir.dt.float32
F32R = mybir.dt.float32r

_CACHE = {}


def build_nc():
    nc = bacc.Bacc("TRN2", target_bir_lowering=False, debug=False)

    xT = nc.dram_tensor("xT", [D_MODEL, N], F32R, kind="ExternalInput")
    wqT = nc.dram_tensor("wqT", [D_MODEL, C_LOC], F32R, kind="ExternalInput")
    wkT = nc.dram_tensor("wkT", [D_MODEL, C_LOC], F32R, kind="ExternalInput")
    wvT = nc.dram_tensor("wvT", [D_MODEL, C_LOC], F32R, kind="ExternalInput")
    woT = nc.dram_tensor("woT", [C_LOC, D_MODEL], F32R, kind="ExternalInput")
    bq = nc.dram_tensor("bq", [C_LOC], F32, kind="ExternalInput")
    bk = nc.dram_tensor("bk", [C_LOC], F32, kind="ExternalInput")
    y = nc.dram_tensor("y", [N, D_MODEL], F32, kind="ExternalOutput")

    NT = N // 128          # 16 k tiles
    DC = D_MODEL // 128    # 8 contraction chunks
    CT = C_LOC // 128      # 2 local-channel tiles
    QW = 512               # q window width

    with tile.TileContext(nc) as tc:
        with (
            tc.tile_pool(name="singles", bufs=1) as singles,
            tc.tile_pool(name="pt", bufs=3) as ptp,
            tc.tile_pool(name="ovs", bufs=4) as ovsp,
            tc.tile_pool(name="norm", bufs=3) as normp,
            tc.tile_pool(name="yout", bufs=2) as youtp,
        ):
            # weights/biases first so the first projection matmuls start early
            bq_sb = singles.tile([128, CT], F32)
            nc.sync.dma_start(bq_sb[:], bq.ap().rearrange("(c p) -> p c", p=128))
            bk_sb = singles.tile([128, CT], F32)
            nc.sync.dma_start(bk_sb[:], bk.ap().rearrange("(c p) -> p c", p=128))
            wqT_sb = singles.tile([128, DC, C_LOC], F32R)
            nc.sync.dma_start(wqT_sb[:], wqT.ap().rearrange("(c p) m -> p c m", p=128))
            wkT_sb = singles.tile([128, DC, C_LOC], F32R)
            nc.sync.dma_start(wkT_sb[:], wkT.ap().rearrange("(c p) m -> p c m", p=128))
            wvT_sb = singles.tile([128, DC, C_LOC], F32R)
            nc.sync.dma_start(wvT_sb[:], wvT.ap().rearrange("(c p) m -> p c m", p=128))
            woT_sb = singles.tile([128, CT, D_MODEL], F32R)
            nc.sync.dma_start(woT_sb[:], woT.ap().rearrange("(c p) m -> p c m", p=128))
            xT_sb = singles.tile([128, DC, N], F32R)
            xT_r = xT.ap().rearrange("(c p) n -> p c n", p=128)
            for dc in range(DC):
                nc.sync.dma_start(xT_sb[:, dc], xT_r[:, dc])

            qT_sb = singles.tile([128, CT, N], F32R)
            kT_sb = singles.tile([128, CT, N], F32R)
            # V per k-tile per head, with a trailing ones column (col 64)
            v1_sb = singles.tile([128, NT, HEADS_PER_CORE, HEAD_DIM + 1], F32R)
            ones_sb = singles.tile([128, NT * HEADS_PER_CORE], F32)
            nc.vector.memset(ones_sb[:], 1.0)
            nc.vector.tensor_copy(
                v1_sb[:, :, :, HEAD_DIM],
                ones_sb[:].rearrange("p (n h) -> p n h", h=HEADS_PER_CORE),
            )
            outT_sb = singles.tile([128, CT, N], F32R)
            onescol = singles.tile([1, 64], F32R)
            nc.vector.tensor_copy(onescol[:], ones_sb[0:1, 0:64])

            # ---- projections.  qT[c, n] = sum_d wqT[d, c] xT[d, n] (+bias);
            # order: q/k of c-tile 0, V, q/k of c-tile 1 -- so head pair 0 can
            # start attention while the rest projects.
            def qk_proj(w_sb, b_sb, dst, ct, pps):
                for nch in range(N // 512):
                    ps = pps.tile([128, 512], F32, tag="qk")
                    for dc in range(DC):
                        nc.tensor.matmul(
                            ps[:],
                            w_sb[:, dc, ct * 128:(ct + 1) * 128],
                            xT_sb[:, dc, nch * 512:(nch + 1) * 512],
                            start=(dc == 0),
                            stop=(dc == DC - 1),
                        )
                    nc.vector.tensor_tensor(
                        dst[:, ct, nch * 512:(nch + 1) * 512],
                        ps[:],
                        b_sb[:, ct:ct + 1].to_broadcast((128, 512)),
                        mybir.AluOpType.add,
                    )


            def make_v_proj_tile(pps):
                # V natural: v[n, c] = sum_d xT[d, n] wvT[d, c], one n tile
                def v_proj_tile(nt):
                    ps = pps.tile([128, 512], F32, tag="qk")
                    for dc in range(DC):
                        nc.tensor.matmul(
                            ps[:, :C_LOC],
                            xT_sb[:, dc, nt * 128:(nt + 1) * 128],
                            wvT_sb[:, dc, :],
                            start=(dc == 0),
                            stop=(dc == DC - 1),
                        )
                    nc.vector.tensor_copy(
                        v1_sb[:, nt, :, 0:HEAD_DIM],
                        ps[:, :C_LOC].rearrange("p (h d) -> p h d",
                                                h=HEADS_PER_CORE),
                    )
                return v_proj_tile

            with (
                tc.tile_pool(name="proj_ps", bufs=2, space="PSUM") as pps,
                tc.tile_pool(name="st_ps", bufs=2, space="PSUM") as stp,
                tc.tile_pool(name="ovy_ps", bufs=2, space="PSUM") as ovp,
                tc.tile_pool(name="dsc", bufs=4, space="DRAM") as dscp,
            ):
                def attention_pair(qq, pair, vproj_interleave=False):
                    q0 = qq * QW
                    ct_h = pair
                    ovA = ovp.tile([HEAD_DIM + 1, QW], F32, tag="ov")
                    ovB = ovp.tile([HEAD_DIM + 1, QW], F32, tag="ov")
                    for kt in range(NT):
                        if vproj_interleave:
                            v_proj_tile(kt)
                        st = stp.tile([128, 2 * QW], F32)
                        for half, p0 in ((0, 0), (1, 64)):
                            nc.tensor.matmul(
                                st[:, half * QW:(half + 1) * QW],
                                kT_sb[p0:p0 + 64, ct_h,
                                      kt * 128:(kt + 1) * 128],
                                qT_sb[p0:p0 + 64, ct_h, q0:q0 + QW],
                                start=True,
                                stop=True,
                            )
                        pt = ptp.tile([128, 2 * QW], F32R)
                        nc.scalar.activation(
                            pt[:], st[:], mybir.ActivationFunctionType.Exp
                        )
                        for half, ov in ((0, ovA), (1, ovB)):
                            nc.tensor.matmul(
                                ov[:],
                                v1_sb[:, kt, 2 * pair + half, :],
                                pt[:, half * QW:(half + 1) * QW],
                                start=(kt == 0),
                                stop=(kt == NT - 1),
                            )
                    # copy accumulators out of PSUM fast, then normalize
                    for half, ov in ((0, ovA), (1, ovB)):
                        p0 = 64 * half
                        ovs = ovsp.tile([HEAD_DIM + 1, QW], F32, tag="ovs")
                        nc.vector.tensor_copy(ovs[:], ov[:])
                        rec = normp.tile([1, QW], F32, tag="nrm")
                        nc.vector.reciprocal(
                            rec[:], ovs[HEAD_DIM:HEAD_DIM + 1, :]
                        )
                        rdram = dscp.tile([1, QW], F32)
                        nc.sync.dma_start(rdram[:], rec[:])
                        bc = normp.tile([64, QW], F32, tag="nrm")
                        nc.gpsimd.dma_start(
                            bc[:], rdram[:].partition_broadcast(64)
                        )
                        nc.vector.tensor_mul(
                            outT_sb[p0:p0 + 64, ct_h, q0:q0 + QW],
                            ovs[0:HEAD_DIM, :],
                            bc[:],
                        )

                def y_proj(qq):
                    # output projection for one finished q window
                    for nt in range(qq * QW // 128, (qq + 1) * QW // 128):
                        for cok in range(D_MODEL // 512):
                            ps = ovp.tile([128, 512], F32, tag="ov")
                            for ct in range(CT):
                                nc.tensor.matmul(
                                    ps[:],
                                    outT_sb[:, ct, nt * 128:(nt + 1) * 128],
                                    woT_sb[:, ct, cok * 512:(cok + 1) * 512],
                                    start=(ct == 0),
                                    stop=(ct == CT - 1),
                                )
                            ys = youtp.tile([128, 512], F32, tag="ys")
                            nc.vector.tensor_copy(ys[:], ps[:])
                            nc.sync.dma_start(
                                y.ap()[nt * 128:(nt + 1) * 128,
                                       cok * 512:(cok + 1) * 512],
                                ys[:],
                            )

                # interleaved emission: attention starts as soon as c-tile 0
                # of q/k is projected; y projection trails one window so its
                # matmuls never head-of-line block the PE stream.
                v_proj_tile = make_v_proj_tile(pps)
                qk_proj(wqT_sb, bq_sb, qT_sb, 0, pps)
                qk_proj(wkT_sb, bk_sb, kT_sb, 0, pps)
                for _nt in range(NT):
                    v_proj_tile(_nt)
                attention_pair(0, 0)
                qk_proj(wqT_sb, bq_sb, qT_sb, 1, pps)
                qk_proj(wkT_sb, bk_sb, kT_sb, 1, pps)
                attention_pair(0, 1)
                for qq in range(1, N // QW):
                    attention_pair(qq, 0)
                    attention_pair(qq, 1)
                    y_proj(qq - 1)
                y_proj(N // QW - 1)

    nc.compile()
    return nc


def kernel(x, Wq, bq, Wk, bk, Wv, bv, Wo, bo):
    x = np.asarray(x, dtype=np.float32)
    Wq = np.asarray(Wq, dtype=np.float32)
    Wk = np.asarray(Wk, dtype=np.float32)
    Wv = np.asarray(Wv, dtype=np.float32)
    Wo = np.asarray(Wo, dtype=np.float32)
    bq = np.asarray(bq, dtype=np.float32)
    bk = np.asarray(bk, dtype=np.float32)
    bv = np.asarray(bv, dtype=np.float32)
    bo = np.asarray(bo, dtype=np.float32)

    if "nc" not in _CACHE:
        _CACHE["nc"] = build_nc()
    nc = _CACHE["nc"]

    s = 2.0 / np.sqrt(8.0)  # fold bipolar *2 and score scale (1/8 split per side)
    in_maps = []
    for core in range(N_CORES):
        b = core // (N_CORES // B)
        g = core % (N_CORES // B)
        ch = slice(g * C_LOC, (g + 1) * C_LOC)
        in_maps.append({
            "xT": np.ascontiguousarray(x[b].T),
            "wqT": np.ascontiguousarray((s * Wq[ch, :]).T),
            "wkT": np.ascontiguousarray((s * Wk[ch, :]).T),
            "wvT": np.ascontiguousarray(Wv[ch, :].T),
            "woT": np.ascontiguousarray(Wo[:, ch].T),
            "bq": ((2.0 * bq[ch] - 1.0) / np.sqrt(8.0)).astype(np.float32),
            "bk": ((2.0 * bk[ch] - 1.0) / np.sqrt(8.0)).astype(np.float32),
        })

    _CACHE["in_maps"] = in_maps
    res = run_bass_kernel_spmd(nc, in_maps, core_ids=list(range(N_CORES)))

    g_per_b = N_CORES // B
    const = (Wo @ bv + bo).astype(np.float32)  # bv folded through out-proj
    out = np.empty((B, N, D_MODEL), dtype=np.float32)
    for b in range(B):
        acc = res.results[b * g_per_b]["y"].astype(np.float32).copy()
        for g in range(1, g_per_b):
            acc += res.results[b * g_per_b + g]["y"]
        out[b] = acc + const
    return out



# revision 5
# speedup vs baseline: 1.3246x; 1.3246x over previous
"""Bipolar self-attention on 8 Trainium2 NeuronCores.

Sharding: data-parallel over batch (B=2 -> 2 groups of 4 cores), tensor-
parallel over heads within a group (16 heads -> 4 heads/core). Each core:
  - projects its head-slice of Q/K transposed ([c, n] layout) and V natural,
    with the bipolar transform (q-0.5)*2 and the 1/sqrt(Dh) score scale
    folded into the projection weights/biases host-side,
  - computes S^T = Kb Qb^T per head tile-by-tile, exponentiates (softmax
    without max subtraction -- scores are O(10), exp is safe in fp32),
  - multiplies P^T by V augmented with a ones column, so the softmax
    denominator falls out of the same matmul (row 64 of the accumulator),
  - normalizes and applies its slice of the output projection (row-parallel).
Host sums the 4 partial outputs per batch and adds the bias terms.

v2 structure (vs the f32r baseline):
  - all matmul operands in bf16 (same PE rate as f32r at moving>=256, but
    half the HBM input traffic and half the SBUF footprint),
  - Q/K c-tile-0 projections accumulate dc-outer while the x chunks stream
    in, in a scoped 8-bank PSUM pool (2-pass rotation staggers the group
    stops so the PSUM->SBUF bias-adds pipeline instead of serializing),
  - the attention kt-loop is software-pipelined: P*V of tile kt-2 issues
    after scores of tile kt, so the PE never waits on the Exp activation,
  - softmax denominators are batch-transposed through DRAM into a [128, 8]
    tile so one cheap reciprocal covers a head pair (the DVE reciprocal on
    a [1, 512] row costs 3.3us; this costs ~0.1us),
  - V projection, Q/K c-tile-1 projection and the output projection are
    interleaved into the attention loops as per-kt PE filler work with
    explicit deadlines ahead of the S matmuls that consume them.
"""

import numpy as np
import ml_dtypes

import concourse.bass as bass
import concourse.tile as tile
from concourse import bacc, mybir
from concourse.bass_utils import run_bass_kernel_spmd

D_MODEL = 1024
NHEAD = 16
HEAD_DIM = 64
B = 2
N = 2048
N_CORES = 8
HEADS_PER_CORE = NHEAD // (N_CORES // B)  # 4
C_LOC = HEADS_PER_CORE * HEAD_DIM  # 256

F32 = mybir.dt.float32
BF16 = mybir.dt.bfloat16

_CACHE = {}

NT = N // 128          # 16 k tiles
DC = D_MODEL // 128    # 8 contraction chunks
CT = C_LOC // 128      # 2 local-channel tiles
QW = 512               # q window width
NW = N // QW           # 4 q windows


def build_nc():
    nc = bacc.Bacc("TRN2", target_bir_lowering=False, debug=False)

    xT = nc.dram_tensor("xT", [D_MODEL, N], BF16, kind="ExternalInput")
    wqT = nc.dram_tensor("wqT", [D_MODEL, C_LOC], BF16, kind="ExternalInput")
    wkT = nc.dram_tensor("wkT", [D_MODEL, C_LOC], BF16, kind="ExternalInput")
    wvT = nc.dram_tensor("wvT", [D_MODEL, C_LOC], BF16, kind="ExternalInput")
    woT = nc.dram_tensor("woT", [C_LOC, D_MODEL], BF16, kind="ExternalInput")
    bq = nc.dram_tensor("bq", [C_LOC], F32, kind="ExternalInput")
    bk = nc.dram_tensor("bk", [C_LOC], F32, kind="ExternalInput")
    y = nc.dram_tensor("y", [N, D_MODEL], F32, kind="ExternalOutput")

    with tile.TileContext(nc) as tc:
        with (
            tc.tile_pool(name="singles", bufs=1) as singles,
            tc.tile_pool(name="pt", bufs=3) as ptp,
            tc.tile_pool(name="ovs", bufs=4) as ovsp,
            tc.tile_pool(name="norm", bufs=3) as normp,
            tc.tile_pool(name="yout", bufs=2) as youtp,
        ):
            # q/k path loads on the sync HWDGE ring (needed first), v/o
            # weights on the scalar ring so the two streams overlap.
            bq_sb = singles.tile([128, CT], F32)
            nc.sync.dma_start(bq_sb[:], bq.ap().rearrange("(c p) -> p c", p=128))
            bk_sb = singles.tile([128, CT], F32)
            nc.sync.dma_start(bk_sb[:], bk.ap().rearrange("(c p) -> p c", p=128))
            wqT_sb = singles.tile([128, DC, C_LOC], BF16)
            nc.sync.dma_start(wqT_sb[:], wqT.ap().rearrange("(c p) m -> p c m", p=128))
            wkT_sb = singles.tile([128, DC, C_LOC], BF16)
            nc.sync.dma_start(wkT_sb[:], wkT.ap().rearrange("(c p) m -> p c m", p=128))
            xT_sb = singles.tile([128, DC, N], BF16)
            xT_r = xT.ap().rearrange("(c p) n -> p c n", p=128)
            for dc in range(DC):
                nc.sync.dma_start(xT_sb[:, dc], xT_r[:, dc])
            wvT_sb = singles.tile([128, DC, C_LOC], BF16)
            nc.scalar.dma_start(wvT_sb[:], wvT.ap().rearrange("(c p) m -> p c m", p=128))
            woT_sb = singles.tile([128, CT, D_MODEL], BF16)
            nc.scalar.dma_start(woT_sb[:], woT.ap().rearrange("(c p) m -> p c m", p=128))

            qT_sb = singles.tile([128, CT, N], BF16)
            kT_sb = singles.tile([128, CT, N], BF16)
            # V per k-tile per head, with a trailing ones column (col 64)
            v1_sb = singles.tile([128, NT, HEADS_PER_CORE, HEAD_DIM + 1], BF16)
            ones_sb = singles.tile([128, NT * HEADS_PER_CORE], F32)
            nc.vector.memset(ones_sb[:], 1.0)
            nc.vector.tensor_copy(
                v1_sb[:, :, :, HEAD_DIM],
                ones_sb[:].rearrange("p (n h) -> p n h", h=HEADS_PER_CORE),
            )
            outT_sb = singles.tile([128, CT, N], BF16)

            # ---- Q/K c-tile 0: dc-outer accumulation overlapped with the
            # x stream. 8 groups (q n-chunks 0-3, k n-chunks 0-3) live in 8
            # PSUM banks; group g consumes chunks g, g+1, ..., g-1 (cyclic)
            # so the stops stagger and the bias-adds pipeline.
            with tc.tile_pool(name="qk0", bufs=1, space="PSUM") as qk0p:
                gps = [qk0p.tile([128, QW], F32, tag=f"g{g}", name=f"g{g}")
                       for g in range(8)]

                def grp_w(g):  # (weight sbuf, bias sbuf, dst sbuf, n-chunk)
                    if g < 4:
                        return wqT_sb, bq_sb, qT_sb, g
                    return wkT_sb, bk_sb, kT_sb, g - 4

                def grp_mm(g, dc):
                    w_sb, _, _, nch = grp_w(g)
                    nc.tensor.matmul(
                        gps[g][:],
                        w_sb[:, dc, 0:128],
                        xT_sb[:, dc, nch * QW:(nch + 1) * QW],
                        start=(dc == g),
                        stop=(dc == (g - 1) % 8),
                    )

                def grp_bias(g):
                    _, b_sb, dst, nch = grp_w(g)
                    nc.vector.tensor_tensor(
                        dst[:, 0, nch * QW:(nch + 1) * QW],
                        gps[g][:],
                        b_sb[:, 0:1].to_broadcast((128, QW)),
                        mybir.AluOpType.add,
                    )

                for c in range(8):           # pass 1: chunk c serves g <= c
                    for g in range(c + 1):
                        grp_mm(g, c)
                    if c == 7:
                        grp_bias(0)
                for c in range(7):           # pass 2: chunk c serves g > c
                    for g in range(c + 1, 8):
                        grp_mm(g, c)
                    grp_bias(c + 1)

            with (
                tc.tile_pool(name="pp", bufs=1, space="PSUM") as pps,
                tc.tile_pool(name="pp2", bufs=1, space="PSUM") as pp2s,
                tc.tile_pool(name="st", bufs=2, space="PSUM") as stp,
                tc.tile_pool(name="ov", bufs=2, space="PSUM") as ovp,
                tc.tile_pool(name="dsc", bufs=4, space="DRAM") as dscp,
            ):
                def v_proj_tile(nt):
                    # V natural: v[n, c] = sum_d xT[d, n] wvT[d, c]
                    ps = pps.tile([128, QW], F32, tag="pp")
                    for dc in range(DC):
                        nc.tensor.matmul(
                            ps[:, :C_LOC],
                            xT_sb[:, dc, nt * 128:(nt + 1) * 128],
                            wvT_sb[:, dc, :],
                            start=(dc == 0),
                            stop=(dc == DC - 1),
                        )
                    nc.vector.tensor_copy(
                        v1_sb[:, nt, :, 0:HEAD_DIM],
                        ps[:, :C_LOC].rearrange("p (h d) -> p h d",
                                                h=HEADS_PER_CORE),
                    )

                # Q/K c-tile 1: 8 sequential (dst, n-chunk) groups of 8
                # single-matmul filler thunks each; the PSUM tile is taken
                # from a dedicated 1-bank pool when the first thunk runs so
                # the group can stay open across many kt iterations without
                # blocking the v/y pool.
                def qk1_group(w_sb, b_sb, dst, nch):
                    state = {}

                    def mm(dc):
                        if dc == 0:
                            state["ps"] = pp2s.tile(
                                [128, QW], F32, tag="pp2", name="pp2ps")
                        nc.tensor.matmul(
                            state["ps"][:],
                            w_sb[:, dc, 128:256],
                            xT_sb[:, dc, nch * QW:(nch + 1) * QW],
                            start=(dc == 0),
                            stop=(dc == DC - 1),
                        )
                        if dc == DC - 1:
                            nc.vector.tensor_tensor(
                                dst[:, 1, nch * QW:(nch + 1) * QW],
                                state["ps"][:],
                                b_sb[:, 1:2].to_broadcast((128, QW)),
                                mybir.AluOpType.add,
                            )

                    return [lambda dc=dc: mm(dc) for dc in range(DC)]

                def y_unit(qq, nt_i, cok):
                    # output projection for one [128, 512] tile of window qq
                    nt = qq * (QW // 128) + nt_i
                    ps = pps.tile([128, QW], F32, tag="pp")
                    for ct in range(CT):
                        nc.tensor.matmul(
                            ps[:],
                            outT_sb[:, ct, nt * 128:(nt + 1) * 128],
                            woT_sb[:, ct, cok * QW:(cok + 1) * QW],
                            start=(ct == 0),
                            stop=(ct == CT - 1),
                        )
                    ys = youtp.tile([128, QW], F32, tag="ys")
                    nc.vector.tensor_copy(ys[:], ps[:])
                    nc.gpsimd.dma_start(
                        y.ap()[nt * 128:(nt + 1) * 128,
                               cok * QW:(cok + 1) * QW],
                        ys[:],
                    )

                def y_units(qq):
                    return [
                        (lambda nt_i=nt_i, cok=cok: y_unit(qq, nt_i, cok))
                        for nt_i in range(QW // 128)
                        for cok in range(D_MODEL // QW)
                    ]

                def normalize(qq, pair, ovA, ovB):
                    q0 = qq * QW
                    ovsA = ovsp.tile([HEAD_DIM + 1, QW], F32, tag="ovs")
                    nc.vector.tensor_copy(ovsA[:], ovA[:])
                    ovsB = ovsp.tile([HEAD_DIM + 1, QW], F32, tag="ovs")
                    nc.vector.tensor_copy(ovsB[:], ovB[:])
                    dn = dscp.tile([2, QW], F32)
                    nc.sync.dma_start(dn[0:1], ovsA[HEAD_DIM:HEAD_DIM + 1, :])
                    nc.sync.dma_start(dn[1:2], ovsB[HEAD_DIM:HEAD_DIM + 1, :])
                    dn_t = dn[:].rearrange("a q -> (a q)").rearrange(
                        "(p c) -> p c", p=128)
                    g = normp.tile([128, 2 * QW // 128], F32, tag="g")
                    nc.sync.dma_start(g[:], dn_t)
                    g2 = normp.tile([128, 2 * QW // 128], F32, tag="g2")
                    nc.vector.reciprocal(g2[:], g[:])
                    rd = dscp.tile([2, QW], F32)
                    rd_t = rd[:].rearrange("a q -> (a q)").rearrange(
                        "(p c) -> p c", p=128)
                    nc.sync.dma_start(rd_t, g2[:])
                    for half, ovs in ((0, ovsA), (1, ovsB)):
                        bc = normp.tile([64, QW], F32, tag="bc")
                        nc.gpsimd.dma_start(
                            bc[:], rd[half:half + 1].partition_broadcast(64)
                        )
                        nc.vector.tensor_mul(
                            outT_sb[64 * half:64 * half + 64, pair,
                                    q0:q0 + QW],
                            ovs[0:HEAD_DIM, :],
                            bc[:],
                        )

                def spread(units, kt0, kt1):
                    """Distribute thunks over kt range [kt0, kt1] into a
                    per-kt schedule dict-of-lists."""
                    sched = {}
                    nkt = kt1 - kt0 + 1
                    done = 0
                    for i in range(nkt):
                        want = (i + 1) * len(units) // nkt
                        if want > done:
                            sched.setdefault(kt0 + i, []).extend(
                                units[done:want])
                            done = want
                    return sched

                def merge(*scheds):
                    out = {}
                    for s in scheds:
                        for k, v in s.items():
                            out.setdefault(k, []).extend(v)
                    return out

                def attention_pair(qq, pair, sched=None):
                    q0 = qq * QW
                    sched = sched or {}
                    ovA = ovp.tile([HEAD_DIM + 1, QW], F32, tag="ov")
                    ovB = ovp.tile([HEAD_DIM + 1, QW], F32, tag="ov")
                    pts = {}

                    def S(kt):
                        st = stp.tile([128, 2 * QW], F32, tag="st")
                        for half, p0 in ((0, 0), (1, 64)):
                            nc.tensor.matmul(
                                st[:, half * QW:(half + 1) * QW],
                                kT_sb[p0:p0 + 64, pair,
                                      kt * 128:(kt + 1) * 128],
                                qT_sb[p0:p0 + 64, pair, q0:q0 + QW],
                                start=True,
                                stop=True,
                            )
                        pt = ptp.tile([128, 2 * QW], BF16, tag="pt")
                        nc.scalar.activation(
                            pt[:], st[:], mybir.ActivationFunctionType.Exp
                        )
                        pts[kt] = pt

                    def OV(kt):
                        pt = pts.pop(kt)
                        for half, ov in ((0, ovA), (1, ovB)):
                            nc.tensor.matmul(
                                ov[:],
                                v1_sb[:, kt, 2 * pair + half, :],
                                pt[:, half * QW:(half + 1) * QW],
                                start=(kt == 0),
                                stop=(kt == NT - 1),
                            )

                    for kt in range(NT):
                        S(kt)
                        if kt >= 2:
                            OV(kt - 2)
                        for f in sched.get(kt, ()):
                            f()
                    OV(NT - 2)
                    OV(NT - 1)
                    normalize(qq, pair, ovA, ovB)

                k1 = [qk1_group(wkT_sb, bk_sb, kT_sb, n) for n in range(4)]
                q1 = [qk1_group(wqT_sb, bq_sb, qT_sb, n) for n in range(4)]

                v_proj_tile(0)
                v_proj_tile(1)
                # window 0 pair 0: remaining V tiles + qk-ct1 k-n0/q-n0
                # (q-n0 must land before pair(0,1)'s S reads qT ct1 win 0)
                attention_pair(0, 0, merge(
                    spread([lambda nt=nt: v_proj_tile(nt)
                            for nt in range(2, NT)], 0, 13),
                    spread(k1[0], 0, 7),
                    spread(q1[0], 8, 15),
                ))
                # pair(0,1) S(kt) reads kT ct1 chunk kt//4: k-n1 by kt 3,
                # k-n2 by kt 7, k-n3 by kt 11.
                attention_pair(0, 1, merge(
                    spread(k1[1], 0, 2),
                    spread(k1[2], 3, 6),
                    spread(k1[3], 7, 10),
                    spread(q1[1], 11, 15),
                ))
                attention_pair(1, 0, merge(
                    spread(q1[2], 0, 7),
                    spread(q1[3], 8, 15),
                ))
                attention_pair(1, 1, spread(y_units(0), 2, 15))
                y1 = y_units(1)
                attention_pair(2, 0, spread(y1[0:4], 4, 15))
                attention_pair(2, 1, spread(y1[4:8], 0, 15))
                y2 = y_units(2)
                attention_pair(3, 0, spread(y2[0:4], 4, 15))
                attention_pair(3, 1, spread(y2[4:8], 0, 15))
                for u in y_units(3):
                    u()

    nc.compile()
    return nc


def kernel(x, Wq, bq, Wk, bk, Wv, bv, Wo, bo):
    x = np.asarray(x, dtype=np.float32)
    Wq = np.asarray(Wq, dtype=np.float32)
    Wk = np.asarray(Wk, dtype=np.float32)
    Wv = np.asarray(Wv, dtype=np.float32)
    Wo = np.asarray(Wo, dtype=np.float32)
    bq = np.asarray(bq, dtype=np.float32)
    bk = np.asarray(bk, dtype=np.float32)
    bv = np.asarray(bv, dtype=np.float32)
    bo = np.asarray(bo, dtype=np.float32)

    if "nc" not in _CACHE:
        _CACHE["nc"] = build_nc()
    nc = _CACHE["nc"]

    bf16 = ml_dtypes.bfloat16
    s = 2.0 / np.sqrt(8.0)  # fold bipolar *2 and score scale (1/8 split per side)
    in_maps = []
    for core in range(N_CORES):
        b = core // (N_CORES // B)
        g = core % (N_CORES // B)
        ch = slice(g * C_LOC, (g + 1) * C_LOC)
        in_maps.append({
            "xT": np.ascontiguousarray(x[b].T).astype(bf16),
            "wqT": np.ascontiguousarray((s * Wq[ch, :]).T).astype(bf16),
            "wkT": np.ascontiguousarray((s * Wk[ch, :]).T).astype(bf16),
            "wvT": np.ascontiguousarray(Wv[ch, :].T).astype(bf16),
            "woT": np.ascontiguousarray(Wo[:, ch].T).astype(bf16),
            "bq": ((2.0 * bq[ch] - 1.0) / np.sqrt(8.0)).astype(np.float32),
            "bk": ((2.0 * bk[ch] - 1.0) / np.sqrt(8.0)).astype(np.float32),
        })

    _CACHE["in_maps"] = in_maps
    res = run_bass_kernel_spmd(nc, in_maps, core_ids=list(range(N_CORES)))

    g_per_b = N_CORES // B
    const = (Wo @ bv + bo).astype(np.float32)  # bv folded through out-proj
    out = np.empty((B, N, D_MODEL), dtype=np.float32)
    for b in range(B):
        acc = res.results[b * g_per_b]["y"].astype(np.float32).copy()
        for g in range(1, g_per_b):
            acc += res.results[b * g_per_b + g]["y"]
        out[b] = acc + const
    return out
